# revision 74
# baseline (speedup 1.0000x reference)
"""Bass/Trainium2 kernel for a 12-layer GPT-style transformer (nn_BERT).

Strategy: data-parallel over batch (B=8 -> 1 sequence per NeuronCore).
Each core runs all 12 layers on x^T [D=768, S=512] ("transposed"
activation layout, feature dim on partitions).

v2: bf16 matmul datapath (FWL weight loads), f32r residual stream,
softmax denominator folded into the AV matmul via a ones column in V,
LayerNorm rsqrt via exp(-0.5*ln(var+eps)) to stay in the exp ACT table
set, software-pipelined emission (k-outer waves after each LayerNorm,
fc/pr interleave), contiguous host-prearranged weight layouts.

kernel(**inputs) takes the FULL unsharded inputs (as produced by
reference.setup_inputs()) and returns the full [8, 512, 768] output.
"""
import contextlib
import os
import sys
import types

sys.path.insert(0, "/opt/trn_rl_repo")
os.environ.setdefault("JAX_PLATFORMS", "axon")

import numpy as np

import concourse.bass as bass
import concourse.mybir as mybir
import concourse.tile as tile
from concourse import bacc
from concourse import bass_utils

F32 = mybir.dt.float32
F32R = mybir.dt.float32r
BF16 = mybir.dt.bfloat16
AF = mybir.ActivationFunctionType
OP = mybir.AluOpType

B, S, D, H, L, V = 8, 512, 768, 12, 12, 40478
DH = D // H            # 64
DF = 4 * D             # 3072
KC = D // 128          # 6 chunks of the model dim
KF = DF // 128         # 24 chunks of the ffn dim
SC = S // 128          # 4 chunks of the sequence
EPS = 1e-5

N_CORES = 8


def _install_ntff_hook():
    """Register the axon NTFF profiling hook that this image's antenv lacks."""
    if "antenv.axon_hooks" in sys.modules:
        return
    try:
        mod = types.ModuleType("antenv.axon_hooks")
        _h = [None]
        mod.set_axon_ntff_profile_hook = lambda h: _h.__setitem__(0, h)
        mod.get_axon_ntff_profile_hook = lambda: _h[0]
        sys.modules["antenv.axon_hooks"] = mod
        import antenv

        antenv.axon_hooks = mod
        if "/root/.axon_site" not in sys.path:
            sys.path.insert(0, "/root/.axon_site")
        from trn_agent_boot.trn_boot import _ntff_profile_via_ctypes

        mod.set_axon_ntff_profile_hook(
            _ntff_profile_via_ctypes("/opt/axon/libaxon_pjrt.so")
        )
    except Exception:
        pass


def build_program(n_layers=L, taps=()):
    nc = bacc.Bacc("TRN2", target_bir_lowering=False, debug=False,
                   num_devices=N_CORES)

    d = {}
    d["x0r"] = nc.dram_tensor("x0r", (D, S), F32R, kind="ExternalInput").ap()
    d["x0b"] = nc.dram_tensor("x0b", (D, S), BF16, kind="ExternalInput").ap()
    d["wqkv"] = nc.dram_tensor("wqkv", (n_layers, 128, KC * 3 * D), BF16,
                               kind="ExternalInput").ap()
    d["wproj"] = nc.dram_tensor("wproj", (n_layers, 128, KC * D), BF16,
                                kind="ExternalInput").ap()
    d["wfca"] = nc.dram_tensor("wfca", (n_layers, 128, KC * D), BF16,
                               kind="ExternalInput").ap()
    d["wfcb"] = nc.dram_tensor("wfcb", (n_layers, KF - KC, 128, KC * 128),
                               BF16, kind="ExternalInput").ap()
    d["wpr"] = nc.dram_tensor("wpr", (n_layers, DF, D), BF16,
                              kind="ExternalInput").ap()
    # biases / gains pre-transposed on host to [L, 128, n]
    d["bqkv"] = nc.dram_tensor("bqkv", (n_layers, 128, 3 * D // 128), F32,
                               kind="ExternalInput").ap()
    d["bv"] = nc.dram_tensor("bv", (n_layers, D), BF16, kind="ExternalInput").ap()
    d["bproj"] = nc.dram_tensor("bproj", (n_layers, 128, KC), F32,
                                kind="ExternalInput").ap()
    d["bfc"] = nc.dram_tensor("bfc", (n_layers, 128, KF), F32,
                              kind="ExternalInput").ap()
    d["bpr"] = nc.dram_tensor("bpr", (n_layers, 128, KC), F32,
                              kind="ExternalInput").ap()
    for nm in ("g1", "b1", "g2", "b2"):
        d[nm] = nc.dram_tensor(nm, (n_layers, 128, KC), F32,
                               kind="ExternalInput").ap()
    d["triu"] = nc.dram_tensor("triu", (128, 128), BF16, kind="ExternalInput").ap()
    d["ones2d_s"] = nc.dram_tensor("ones2d_s", (128, 128), F32R,
                                   kind="ExternalInput").ap()
    d["ones_red_s"] = nc.dram_tensor("ones_red_s", (128, 2), F32R,
                                     kind="ExternalInput").ap()
    d["ones1"] = nc.dram_tensor("ones1", (1, 128), F32R, kind="ExternalInput").ap()
    d["sel1"] = nc.dram_tensor("sel1", (1, 64), F32R,
                               kind="ExternalInput").ap()
    d["out"] = nc.dram_tensor("out", (D, S), F32R, kind="ExternalOutput").ap()
    d["warmo"] = nc.dram_tensor("warmo", (1, 1), F32, kind="ExternalOutput").ap()
    d["_taps"] = {}
    for tp in taps:
        if tp.startswith("v"):
            shape = (SC, 128, H, 66)
        else:
            shape = (KC, 128, S)
        d["_taps"][tp] = nc.dram_tensor(
            f"tap_{tp}", shape, BF16 if tp in ("q", "k", "v", "a", "n1") else F32R,
            kind="ExternalOutput").ap()

    with tile.TileContext(nc) as tc, \
         nc.allow_low_precision(reason="bf16 matmul datapath; rel-err budget 2e-2"):
        _emit(tc, nc, n_layers, d)
    nc.compile()
    return nc


def _tap(nc, d, name, tiles):
    if name in d["_taps"]:
        for i, t in enumerate(tiles):
            nc.sync.dma_start(out=d["_taps"][name][i], in_=t)


def _emit(tc, nc, n_layers, d):
    ctx = contextlib.ExitStack()

    consts = ctx.enter_context(tc.tile_pool(name="consts", bufs=1))
    persist = ctx.enter_context(tc.tile_pool(name="persist", bufs=1))
    resid = ctx.enter_context(tc.tile_pool(name="resid", bufs=2))
    xb_pool = ctx.enter_context(tc.tile_pool(name="xb", bufs=2))
    wq_pool = ctx.enter_context(tc.tile_pool(name="wq", bufs=1))
    wp_pool = ctx.enter_context(tc.tile_pool(name="wp", bufs=1))
    wf_pool = ctx.enter_context(tc.tile_pool(name="wf", bufs=1))
    wr_pool = ctx.enter_context(tc.tile_pool(name="wr", bufs=4))
    wfb_pool = ctx.enter_context(tc.tile_pool(name="wfb", bufs=4))
    probs_pool = ctx.enter_context(tc.tile_pool(name="probs", bufs=2))
    gelu_pool = ctx.enter_context(tc.tile_pool(name="gelu", bufs=8))
    stats_pool = ctx.enter_context(tc.tile_pool(name="stats", bufs=2))
    small_pool = ctx.enter_context(tc.tile_pool(name="small", bufs=2))
    lnsm_pool = ctx.enter_context(tc.tile_pool(name="lnsm", bufs=1))
    lnbc_pool = ctx.enter_context(tc.tile_pool(name="lnbc", bufs=2))
    bias_pool = ctx.enter_context(tc.tile_pool(name="bias", bufs=2))

    cn = {}
    for nm, shape, dt_ in (
        ("triu", [128, 128], BF16),
        ("ones2d_s", [128, 128], F32R),
        ("ones_red_s", [128, 2], F32R),
        ("ones1", [1, 128], F32R),
        ("sel1", [1, 64], F32R),
    ):
        cn[nm] = consts.tile(shape, dt_, tag=nm, name=nm)
        nc.sync.dma_start(out=cn[nm], in_=d[nm])
    cn["eps"] = consts.tile([1, 1], F32, tag="eps", name="eps")
    nc.vector.memset(cn["eps"], EPS)
    # warm-up chain for ACT table preloads: each op reads+writes this tile
    # so the chain stays live (a dead output would be DCE'd by walrus)
    cn["warm"] = consts.tile([1, 1], F32, tag="warm", name="warm")
    nc.vector.memset(cn["warm"], 0.25)


    # v in natural layout [seq, head, dh] with a trailing ones column:
    # lhsT slice [:, h, 1:66] -> out rows 0:63 = v, row 64 = sum(probs)
    # (the softmax denominator rides the AV matmul for free).
    v_nat = [persist.tile([128, H, 66], BF16, tag=f"vn{c}", name=f"vn{c}")
             for c in range(SC)]
    for c in range(SC):
        nc.gpsimd.memset(v_nat[c][:, :, 65:66], 1.0)

    qT = [persist.tile([128, S], BF16, tag=f"q{i}", name=f"qT{i}")
          for i in range(KC)]
    kT = [persist.tile([128, S], BF16, tag=f"k{i}", name=f"kT{i}")
          for i in range(KC)]
    aT = [persist.tile([128, S], BF16, tag=f"a{i}", name=f"aT{i}")
          for i in range(KC)]

    pools = dict(resid=resid, xb=xb_pool, wq=wq_pool, wp=wp_pool, wf=wf_pool,
                 wr=wr_pool, wfb=wfb_pool, probs=probs_pool, gelu=gelu_pool,
                 stats=stats_pool, small=small_pool, lnsm=lnsm_pool,
                 lnbc=lnbc_pool, bias=bias_pool, persist=persist,
                 v_nat=v_nat, qT=qT, kT=kT, aT=aT)

    # layer 0 inputs + weights
    wq_t = wq_pool.tile([128, KC * 3 * D], BF16, tag="wqkv", name="wq0")
    nc.sync.dma_start(out=wq_t, in_=d["wqkv"][0])
    wfc_t = wf_pool.tile([128, KC * D], BF16, tag="wfca", name="wfca0")
    nc.sync.dma_start(out=wfc_t, in_=d["wfca"][0])

    xb = [xb_pool.tile([128, S], BF16, tag=f"xb{k}", name=f"xb0_{k}")
          for k in range(KC)]
    xr = [resid.tile([128, S], F32R, tag=f"r{k}", name=f"xr0_{k}")
          for k in range(KC)]
    for k in range(KC):
        nc.sync.dma_start(out=xb[k], in_=d["x0b"][128 * k:128 * (k + 1), :])
        nc.sync.dma_start(out=xr[k], in_=d["x0r"][128 * k:128 * (k + 1), :])

    feed = None   # per-chunk emit callbacks from previous layer's LN2
    for l in range(n_layers):
        with nc.named_scope(f"layer{l}"):
            wq_t, wfc_t, xb, xr, feed = _layer(
                tc, nc, l, n_layers, wq_t, wfc_t, xb, xr, feed, d, cn, pools)

    # final LN2 apply chunks (last layer's feed) then store
    for k in range(KC):
        feed[k]()
        nc.sync.dma_start(out=d["out"][128 * k:128 * (k + 1), :], in_=xr[k])
    nc.sync.dma_start(out=d["warmo"], in_=cn["warm"])
    ctx.close()


class _LNState:
    """LayerNorm over the partition (feature) axis, pipelined:
    accum() per chunk (PE reductions), finish() (var chain + SBUF
    snapshots of mean/rsd broadcasts), apply() per chunk
    (normalize+affine, dual bf16/f32r outputs).

    All PSUM pools open and close inside finish()/__init__, strictly
    nested within the caller's enclosing PSUM phase pool, so the tile
    allocator's stack discipline holds.  apply() reads only SBUF.
    """

    def __init__(self, tc, nc, tag, cn, stats, lnsm, lnbc):
        self.tc, self.nc, self.cn = tc, nc, cn
        self.stats, self.small, self.lnbc, self.tag = stats, lnsm, lnbc, tag
        self.ctx = contextlib.ExitStack()
        ps_mu = self.ctx.enter_context(
            tc.tile_pool(name=f"{tag}_mu", bufs=1, space="PSUM"))
        self._psq_ctx = contextlib.ExitStack()
        ps_q = self._psq_ctx.enter_context(
            tc.tile_pool(name=f"{tag}_q", bufs=1, space="PSUM"))
        self.mu = ps_mu.tile([128, S], F32, tag="mu", name=f"{tag}_mu")
        self.psq = ps_q.tile([2, S], F32, tag="sq", name=f"{tag}_sq")

    def accum(self, src, k):
        nc, cn = self.nc, self.cn
        sq = self.stats.tile([128, S], F32R, tag="sq", name=f"{self.tag}sq{k}")
        nc.gpsimd.tensor_tensor(out=sq, in0=src, in1=src, op=OP.mult)
        nc.tensor.matmul(self.mu, cn["ones2d_s"], src,
                         start=(k == 0), stop=(k == KC - 1))
        nc.tensor.matmul(self.psq, cn["ones_red_s"], sq,
                         start=(k == 0), stop=(k == KC - 1))

    def finish(self, ext_prs=False):
        nc, cn = self.nc, self.cn
        var = self.small.tile([1, S], F32, tag="var", name=f"{self.tag}var")
        nc.scalar.activation(out=var, in_=self.mu[0:1, :], func=AF.Square)
        v2 = self.small.tile([1, S], F32, tag="v2", name=f"{self.tag}v2")
        nc.vector.tensor_tensor(out=v2, in0=self.psq[0:1, :], in1=var,
                                op=OP.subtract)
        # sqrt table was pre-warmed by a dummy activation, so this is cheap
        sd = self.small.tile([1, S], F32, tag="var", name=f"{self.tag}sd")
        nc.scalar.activation(out=sd, in_=v2, func=AF.Sqrt, bias=cn["eps"])
        rc = self.small.tile([1, S], F32, tag="v2", name=f"{self.tag}rc")
        nc.vector.reciprocal_approx_fast(out=rc, in_=sd)
        rsd = self.small.tile([1, S], F32R, tag="rsd", name=f"{self.tag}rsd")
        nc.vector.tensor_copy(out=rsd, in_=rc)
        self.mu_s = self.lnbc.tile([128, S], F32, tag="mu_s",
                                   name=f"{self.tag}_mus")
        nc.scalar.activation(out=self.mu_s, in_=self.mu, func=AF.Identity)
        self._psq_ctx.close()          # free the psq bank
        self.rsd = rsd
        if not ext_prs:
            with self.tc.tile_pool(name=f"{self.tag}_prs", bufs=1,
                                   space="PSUM") as ps_prs:
                prs = ps_prs.tile([128, S], F32, tag="prs",
                                  name=f"{self.tag}_prs")
                nc.tensor.matmul(prs, cn["ones1"], rsd, start=True, stop=True)
                self.prs_s = self.lnbc.tile([128, S], F32, tag="prs_s",
                                            name=f"{self.tag}_prss")
                nc.scalar.activation(out=self.prs_s, in_=prs,
                                     func=AF.Identity)
        self.ctx.close()               # free the mu bank

    def set_prs(self, prs):
        """Emit the rsd broadcast into a caller-owned PSUM tile; apply()
        then reads it directly (saves the SBUF snapshot copy)."""
        self.nc.tensor.matmul(prs, self.cn["ones1"], self.rsd,
                              start=True, stop=True)
        self.prs_s = prs

    def apply(self, k, src, g_t, b_t, dst_bf, dst_r):
        nc = self.nc
        t = self.stats.tile([128, S], F32, tag="t", name=f"{self.tag}t{k}")
        nc.gpsimd.tensor_tensor(out=t, in0=src, in1=self.mu_s,
                                op=OP.subtract)
        u = self.stats.tile([128, S], F32, tag="u", name=f"{self.tag}u{k}")
        nc.vector.scalar_tensor_tensor(out=u, in0=t, scalar=g_t[:, k:k + 1],
                                       in1=self.prs_s, op0=OP.mult,
                                       op1=OP.mult)
        nc.scalar.activation(out=dst_bf, in_=u, func=AF.Identity,
                             bias=b_t[:, k:k + 1], scale=1.0)
        nc.scalar.activation(out=dst_r, in_=u, func=AF.Identity,
                             bias=b_t[:, k:k + 1], scale=1.0)


def _layer(tc, nc, l, n_layers, wq_t, wfc_t, xb, xr, feed, d, cn, pools):
    bias_pool = pools["bias"]
    stats = pools["stats"]
    small = pools["small"]
    lnsm = pools["lnsm"]
    v_nat, qT, kT, aT = pools["v_nat"], pools["qT"], pools["kT"], pools["aT"]

    # ---- layer-start weight/bias DMAs (wproj single-buffered: its
    # previous-layer reads finished phases ago, so this runs during A). ----
    wproj_t = pools["wp"].tile([128, KC * D], BF16, tag="wproj")
    nc.sync.dma_start(out=wproj_t, in_=d["wproj"][l])

    def ld_bias(name, width):
        t = bias_pool.tile([128, width], F32, tag=name, name=f"{name}{l}")
        nc.sync.dma_start(out=t, in_=d[name][l])
        return t

    bqkv_t = ld_bias("bqkv", 3 * D // 128)
    bproj_t = ld_bias("bproj", KC)
    g1_t = ld_bias("g1", KC)
    b1_t = ld_bias("b1", KC)
    bfc_t = ld_bias("bfc", KF)
    bpr_t = ld_bias("bpr", KC)
    g2_t = ld_bias("g2", KC)
    b2_t = ld_bias("b2", KC)
    bv_b = bias_pool.tile([128, D], BF16, tag="bvb", name=f"bvb{l}")
    nc.sync.dma_start(out=bv_b, in_=d["bv"][l].partition_broadcast(128))
    bv_h = bv_b.rearrange("p (h e) -> p h e", e=DH)

    # =====================================================================
    # Phase A.  wave0: k-outer over x chunks as LN2(l-1) produces them:
    # v half0 (4 psum groups) + qkv oc 0,1 (2 groups) = 6 banks, plus the
    # still-live LN2 mu/prs banks of the previous layer = 8.
    # =====================================================================
    VOFF = 1536  # column offset of v inside wqkv's 2304-wide block

    # wave0: all 8 v psum groups (LN2(l-1) psum pools are fully closed, so
    # all 8 banks are free); 3072 rows per x-chunk matches the feed rate
    with tc.tile_pool(name="ps_w0", bufs=1, space="PSUM") as ps_w0:
        pv0 = [ps_w0.tile([128, 384], F32, tag=f"pv0_{j}", name=f"pv0_{j}")
               for j in range(2 * SC)]
        for k in range(KC):
            if feed is not None:
                feed[k]()          # emit LN2(l-1) apply for chunk k
            for sc in range(SC):
                for half in range(2):
                    nc.tensor.matmul(
                        pv0[2 * sc + half], xb[k][:, 128 * sc:128 * (sc + 1)],
                        wq_t[:, k * 2304 + VOFF + 384 * half:
                             k * 2304 + VOFF + 384 * (half + 1)],
                        start=(k == 0), stop=(k == KC - 1))
        for sc in range(SC):
            for half in range(2):
                nc.vector.tensor_tensor(
                    out=v_nat[sc][:, 6 * half:6 * half + 6, 1:65],
                    in0=pv0[2 * sc + half].rearrange("p (h e) -> p h e", e=DH),
                    in1=bv_h[:, 6 * half:6 * half + 6, :], op=OP.add)

    # wave1: all q/k output chunks; kT chunks early so phase B starts
    with tc.tile_pool(name="ps_qk", bufs=3, space="PSUM") as ps_qk:
        for oc in [6, 0, 7, 1, 8, 2, 9, 3, 10, 4, 11, 5]:
            pt = ps_qk.tile([128, S], F32, tag="qk")
            for k in range(KC):
                nc.tensor.matmul(
                    pt,
                    wq_t[:, k * 2304 + 128 * oc:k * 2304 + 128 * (oc + 1)],
                    xb[k], start=(k == 0), stop=(k == KC - 1))
            dst = qT[oc] if oc < KC else kT[oc - KC]
            nc.vector.tensor_scalar(out=dst, in0=pt,
                                    scalar1=bqkv_t[:, oc:oc + 1],
                                    scalar2=None, op0=OP.add)

    if l == 0:
        _tap(nc, d, "q", qT)
        _tap(nc, d, "k", kT)
        _tap(nc, d, "v", v_nat)

    # =====================================================================
    # Phase B: attention; softmax denominator rides row 64 of pav, is
    # DMA-gathered into den12, and pairs share one reciprocal.
    # =====================================================================
    with tc.tile_pool(name="ps_sc", bufs=2, space="PSUM") as ps_sc, \
         tc.tile_pool(name="ps_av", bufs=3, space="PSUM") as ps_av, \
         tc.tile_pool(name="ps_dn", bufs=2, space="PSUM") as ps_dn, \
         tc.tile_pool(name="ps_bc", bufs=1, space="PSUM") as ps_bc:

        pavs = {}

        def emit_head(h):
            hc, hh = h // 2, (h % 2) * 64
            probs = [pools["probs"].tile([128, S], BF16, tag=f"pb{c}",
                                         name=f"pb{c}_{h}") for c in range(SC)]
            for c in range(SC):
                n0 = 128 * c
                pt = ps_sc.tile([128, S], F32, tag="score")
                nc.tensor.matmul(pt[:, 0:S - n0],
                                 kT[hc][hh:hh + 64, n0:n0 + 128],
                                 qT[hc][hh:hh + 64, n0:S],
                                 start=True, stop=True)
                nc.scalar.activation(out=probs[c][:, n0:S],
                                     in_=pt[:, 0:S - n0],
                                     func=AF.Exp, scale=0.125)
                nc.gpsimd.tensor_tensor(
                    out=probs[c][:, n0:n0 + 128],
                    in0=probs[c][:, n0:n0 + 128],
                    in1=cn["triu"], op=OP.mult)
            pav = ps_av.tile([64, S], F32, tag="pav", name=f"pav{h}")
            pden = ps_dn.tile([1, S], F32, tag="pden", name=f"pden{h}")
            pavs[h] = (pav, pden)
            for c in range(SC):
                n0 = 128 * c
                nc.tensor.matmul(pav[0:64, n0:S], v_nat[c][:, h, 1:65],
                                 probs[c][:, n0:S], start=(c == 0),
                                 stop=(c == SC - 1), skip_group_check=True)
                nc.tensor.matmul(pden[0:1, n0:S], v_nat[c][:, h, 65:66],
                                 probs[c][:, n0:S], start=(c == 0),
                                 stop=(c == SC - 1), skip_group_check=True)

        def emit_norm(h):
            hc = h // 2
            pav, pden = pavs.pop(h)
            rec = small.tile([1, S], F32, tag="rec", name=f"rec{h}")
            nc.vector.reciprocal_approx_fast(out=rec, in_=pden)
            rec_r = small.tile([1, S], F32R, tag="recr", name=f"recr{h}")
            nc.vector.tensor_copy(out=rec_r, in_=rec)
            pbc = ps_bc.tile([64, S], F32, tag="pbc")
            nc.tensor.matmul(pbc, cn["sel1"], rec_r, start=True, stop=True)
            bc_s = small.tile([64, S], F32, tag="bcs", name=f"bcs{h}")
            nc.vector.tensor_copy(out=bc_s, in_=pbc)
            if h % 2 == 0:
                nc.vector.tensor_tensor(out=aT[hc][0:64, :],
                                        in0=pav[0:64, :], in1=bc_s,
                                        op=OP.mult)
            else:
                att = small.tile([64, S], BF16, tag="att", name=f"att{h}")
                nc.vector.tensor_tensor(out=att, in0=pav[0:64, :],
                                        in1=bc_s, op=OP.mult)
                nc.sync.dma_start(out=aT[hc][64:128, :], in_=att)

        for h in range(H):
            if h >= 2:
                emit_norm(h - 2)
            emit_head(h)
        emit_norm(H - 2)
        emit_norm(H - 1)
        # warm the sqrt table while the PE runs proj (anchored on aT so the
        # scheduler can't hoist it; bias chains warm's liveness)
        nc.scalar.activation(out=cn["warm"], in_=aT[5][0:1, 0:1],
                             func=AF.Sqrt, bias=cn["warm"])

    # prefetch next layer's wqkv now: the burst runs during C/LN1, clear of
    # phase B's latency-critical small DMAs
    wq_next = None
    if l + 1 < n_layers:
        wq_next = pools["wq"].tile([128, KC * 3 * D], BF16, tag="wqkv",
                                   name=f"wq{l + 1}")
        nc.sync.dma_start(out=wq_next, in_=d["wqkv"][l + 1])

    if l == 0:
        _tap(nc, d, "a", aT)

    # =====================================================================
    # Phase C: attn out proj; residual+bias via one DVE STT; LN1 stats
    # interleaved so the PE stays busy into the LN tail.
    # =====================================================================
    res1 = [pools["resid"].tile([128, S], F32R, tag=f"r{k}", name=f"res1_{k}")
            for k in range(KC)]

    with tc.tile_pool(name="ps_pj", bufs=3, space="PSUM") as ps_pj:
        ln1 = _LNState(tc, nc, "ln1", cn, stats, lnsm, pools["lnbc"])
        for oc in range(KC):
            pt = ps_pj.tile([128, S], F32, tag="pj")
            for k in range(KC):
                nc.tensor.matmul(
                    pt, wproj_t[:, k * D + 128 * oc:k * D + 128 * (oc + 1)],
                    aT[k], start=(k == 0), stop=(k == KC - 1))
            nc.vector.scalar_tensor_tensor(out=res1[oc], in0=pt,
                                           scalar=bproj_t[:, oc:oc + 1],
                                           in1=xr[oc], op0=OP.add, op1=OP.add)
            ln1.accum(res1[oc], oc)
        ln1.finish(ext_prs=True)
        # warm the gelu table while the PE runs the fc wave
        nc.scalar.activation(out=cn["warm"], in_=ln1.rsd[0:1, 0:1],
                             func=AF.Gelu_apprx_tanh, bias=cn["warm"])
    if l == 0:
        _tap(nc, d, "r1", res1)

    # =====================================================================
    # Phase D.  LN1 apply feeds fc wave0 (k-outer, 6 psum groups), then
    # fc(kf)/pr(kf-6) interleaved so the PE never waits on gelu.
    # =====================================================================
    nT_b = [pools["xb"].tile([128, S], BF16, tag=f"xb{k}", name=f"nTb{k}")
            for k in range(KC)]
    nT_r = [pools["resid"].tile([128, S], F32R, tag=f"r{k}", name=f"nTr{k}")
            for k in range(KC)]
    gks = [pools["gelu"].tile([128, S], BF16, tag="gk", name=f"gk{kf}")
           for kf in range(KF)]

    with tc.tile_pool(name="ps_f0", bufs=1, space="PSUM") as ps_f0:
        pf0 = [ps_f0.tile([128, S], F32, tag=f"pf{j}", name=f"pf{j}")
               for j in range(KC)]
        ln1_prs = ps_f0.tile([128, S], F32, tag="ln1prs", name="ln1prs")
        ln1.set_prs(ln1_prs)
        for k in range(KC):
            ln1.apply(k, res1[k], g1_t, b1_t, nT_b[k], nT_r[k])
            for j in range(KC):
                nc.tensor.matmul(
                    pf0[j], wfc_t[:, k * D + 128 * j:k * D + 128 * (j + 1)],
                    nT_b[k], start=(k == 0), stop=(k == KC - 1))
        for j in range(KC):
            nc.scalar.activation(out=gks[j], in_=pf0[j],
                                 func=AF.Gelu_apprx_tanh,
                                 bias=bfc_t[:, j:j + 1], scale=1.0)
    if l == 0:
        _tap(nc, d, "n1", nT_b)

    res2 = [pools["resid"].tile([128, S], F32R, tag=f"r{k}", name=f"res2_{k}")
            for k in range(KC)]
    ln2_box = [None]

    with tc.tile_pool(name="ps_pr", bufs=1, space="PSUM") as ps_pr:
        pr_acc = [ps_pr.tile([128, S], F32, tag=f"pr{oc}", name=f"pr{oc}")
                  for oc in range(KC)]
        wprs = {}

        def dma_wpr(kf):
            wpr_k = pools["wr"].tile([128, D], BF16, tag="wprk")
            nc.sync.dma_start(out=wpr_k,
                              in_=d["wpr"][l, 128 * kf:128 * (kf + 1), :])
            wprs[kf] = wpr_k

        wfbs = {}

        def dma_wfcb(kf):
            wfb = pools["wfb"].tile([128, KC * 128], BF16, tag="wfcb")
            nc.sync.dma_start(out=wfb, in_=d["wfcb"][l, kf - KC])
            wfbs[kf] = wfb

        def emit_pr(kf, tail=False):
            wpr_k = wprs.pop(kf)
            for oc in range(KC):
                nc.tensor.matmul(pr_acc[oc],
                                 wpr_k[:, 128 * oc:128 * (oc + 1)],
                                 gks[kf], start=(kf == 0),
                                 stop=(kf == KF - 1))
                if tail:
                    nc.vector.scalar_tensor_tensor(
                        out=res2[oc], in0=pr_acc[oc],
                        scalar=bpr_t[:, oc:oc + 1], in1=nT_r[oc],
                        op0=OP.add, op1=OP.add)
                    ln2_box[0].accum(res2[oc], oc)

        with tc.tile_pool(name="ps_fc", bufs=2, space="PSUM") as ps_fc:
            for j in range(4):
                dma_wpr(j)
            for j in range(4):
                dma_wfcb(KC + j)
            for kf in range(KC, KF):
                wfb = wfbs.pop(kf)
                pf = ps_fc.tile([128, S], F32, tag="fc")
                for k in range(KC):
                    nc.tensor.matmul(
                        pf, wfb[:, 128 * k:128 * (k + 1)],
                        nT_b[k], start=(k == 0), stop=(k == KC - 1))
                nc.scalar.activation(out=gks[kf], in_=pf,
                                     func=AF.Gelu_apprx_tanh,
                                     bias=bfc_t[:, kf:kf + 1], scale=1.0)
                emit_pr(kf - KC)
                dma_wpr(kf - 2)
                if kf + 4 < KF:
                    dma_wfcb(kf + 4)
            # warm the sqrt table while the PE runs the pr tail (anchored on
            # the last gelu output so it can't run before the gelu stream)
            nc.scalar.activation(out=cn["warm"], in_=gks[KF - 1][0:1, 0:1],
                                 func=AF.Sqrt, bias=cn["warm"])
        for kf in range(KF - KC, KF - 1):
            if kf + 4 < KF:
                dma_wpr(kf + 4)
            emit_pr(kf)
        # ps_fc closed -> 2 banks free for LN2 mu/psq during the tail
        ln2_box[0] = _LNState(tc, nc, "ln2", cn, stats, lnsm, pools["lnbc"])
        emit_pr(KF - 1, tail=True)
        ln2_box[0].finish()
        # warm the exp table for the next layer's softmax (anchored on the
        # LN2 sqrt-chain output so it cannot evict the sqrt table early)
        nc.scalar.activation(out=cn["warm"], in_=ln2_box[0].prs_s[0:1, 0:1],
                             func=AF.Exp, bias=cn["warm"])
    ln2 = ln2_box[0]
    if l == 0:
        _tap(nc, d, "r2", res2)

    # prefetch next layer's wfc at the END of the layer: the 13us burst
    # lands behind this layer's wpr stream and runs during LN2/A/B of l+1
    wfc_next = None
    if l + 1 < n_layers:
        wfc_next = pools["wf"].tile([128, KC * D], BF16, tag="wfca",
                                    name=f"wfca{l + 1}")
        nc.sync.dma_start(out=wfc_next, in_=d["wfca"][l + 1])

    xb_new = [pools["xb"].tile([128, S], BF16, tag=f"xb{k}", name=f"xbn{k}")
              for k in range(KC)]
    xr_new = [pools["resid"].tile([128, S], F32R, tag=f"r{k}", name=f"xrn{k}")
              for k in range(KC)]

    def make_feed(k):
        def f():
            ln2.apply(k, res2[k], g2_t, b2_t, xb_new[k], xr_new[k])
        return f

    feed_new = [make_feed(k) for k in range(KC)]
    return wq_next, wfc_next, xb_new, xr_new, feed_new


# =========================================================================
# Host side
# =========================================================================
_CACHE = {}


def _get_program():
    if "nc" not in _CACHE:
        _install_ntff_hook()
        _CACHE["nc"] = build_program(L)
    return _CACHE["nc"]


def make_in_maps(inputs, n_layers=L):
    import ml_dtypes
    bf16 = ml_dtypes.bfloat16

    tokens = np.asarray(inputs["tokens"])
    we = np.asarray(inputs["we"], dtype=np.float32)
    pos = we[V:V + S]                                  # [S, D]

    def f32(name):
        return np.ascontiguousarray(np.asarray(inputs[name])[:n_layers],
                                    dtype=np.float32)

    def pack(w, n):
        # [L, D, n] -> [L, 128, KC*n]: a[l, p, k*n + j] = w[l, 128k+p, j]
        return np.ascontiguousarray(
            w.reshape(n_layers, KC, 128, n).transpose(0, 2, 1, 3).reshape(
                n_layers, 128, KC * n)).astype(bf16)

    def pack_fca(w):
        # [L, D, DF] -> [L, 128, KC*D]: a[l,p,k*768+j*128+c] = w[l,128k+p,128j+c], j<6
        return np.ascontiguousarray(
            w[:, :, :D].reshape(n_layers, KC, 128, D).transpose(
                0, 2, 1, 3).reshape(n_layers, 128, KC * D)).astype(bf16)

    def pack_fcb(w):
        # [L, D, DF] -> [L, KF-KC, 128, KC*128]:
        # b[l, kf-6, p, k*128+c] = w[l, 128k+p, 128kf+c]
        t = w[:, :, D:].reshape(n_layers, KC, 128, KF - KC, 128)
        return np.ascontiguousarray(
            t.transpose(0, 3, 2, 1, 4).reshape(
                n_layers, KF - KC, 128, KC * 128)).astype(bf16)

    def bias_t(b, n):
        # [L, n*128] -> [L, 128, n]
        return np.ascontiguousarray(
            b.reshape(n_layers, n, 128).transpose(0, 2, 1))

    bqkv = f32("bqkv")
    shared = {
        "wqkv": pack(f32("wqkv"), 3 * D),
        "wproj": pack(f32("wproj"), D),
        "wfca": pack_fca(f32("wfc")),
        "wfcb": pack_fcb(f32("wfc")),
        "wpr": np.ascontiguousarray(f32("wpr")).astype(bf16),
        "bqkv": bias_t(bqkv, 3 * D // 128),
        "bv": np.ascontiguousarray(bqkv[:, 2 * D:3 * D]).astype(bf16),
        "bproj": bias_t(f32("bproj"), KC),
        "bfc": bias_t(f32("bfc"), KF),
        "bpr": bias_t(f32("bpr"), KC),
        "g1": bias_t(f32("g1"), KC),
        "b1": bias_t(f32("b1"), KC),
        "g2": bias_t(f32("g2"), KC),
        "b2": bias_t(f32("b2"), KC),
        "triu": np.triu(np.ones((128, 128), np.float32)).astype(bf16),
        "ones2d_s": np.full((128, 128), 1.0 / D, np.float32),
        "ones1": np.ones((1, 128), np.float32),
    }
    shared["sel1"] = np.ones((1, 64), np.float32)
    ones_red = np.zeros((128, 2), np.float32)
    ones_red[:, 0] = 1.0 / D
    shared["ones_red_s"] = ones_red

    in_maps = []
    for b in range(N_CORES):
        x0 = we[tokens[b]] + pos                       # [S, D]
        x0T = np.ascontiguousarray(x0.T, dtype=np.float32)
        m = dict(shared)
        m["x0r"] = x0T
        m["x0b"] = x0T.astype(bf16)
        in_maps.append(m)
    return in_maps


def run(inputs, trace=False):
    nc = _get_program()
    in_maps = make_in_maps(inputs)
    res = bass_utils.run_bass_kernel_spmd(nc, in_maps,
                                          core_ids=list(range(N_CORES)),
                                          trace=trace)
    outs = np.stack([np.asarray(res.results[b]["out"]).T
                     for b in range(N_CORES)])
    return outs.astype(np.float32), res


def kernel(**inputs):
    out, _ = run(inputs, trace=False)
    return out


# revision 75
# speedup vs baseline: 1.0206x; 1.0206x over previous
"""Bass/Trainium2 kernel for a 12-layer GPT-style transformer (nn_BERT).

Strategy: data-parallel over batch (B=8 -> 1 sequence per NeuronCore).
Each core runs all 12 layers on x^T [D=768, S=512] ("transposed"
activation layout, feature dim on partitions).

v2: bf16 matmul datapath (FWL weight loads), f32r residual stream,
softmax denominator folded into the AV matmul via a ones column in V,
LayerNorm rsqrt via exp(-0.5*ln(var+eps)) to stay in the exp ACT table
set, software-pipelined emission (k-outer waves after each LayerNorm,
fc/pr interleave), contiguous host-prearranged weight layouts.

kernel(**inputs) takes the FULL unsharded inputs (as produced by
reference.setup_inputs()) and returns the full [8, 512, 768] output.
"""
import contextlib
import os
import sys
import types

sys.path.insert(0, "/opt/trn_rl_repo")
os.environ.setdefault("JAX_PLATFORMS", "axon")

import numpy as np

import concourse.bass as bass
import concourse.mybir as mybir
import concourse.tile as tile
from concourse import bacc
from concourse import bass_utils

F32 = mybir.dt.float32
F32R = mybir.dt.float32r
BF16 = mybir.dt.bfloat16
AF = mybir.ActivationFunctionType
OP = mybir.AluOpType

B, S, D, H, L, V = 8, 512, 768, 12, 12, 40478
DH = D // H            # 64
DF = 4 * D             # 3072
KC = D // 128          # 6 chunks of the model dim
KF = DF // 128         # 24 chunks of the ffn dim
SC = S // 128          # 4 chunks of the sequence
EPS = 1e-5

N_CORES = 8


def _install_ntff_hook():
    """Register the axon NTFF profiling hook that this image's antenv lacks."""
    if "antenv.axon_hooks" in sys.modules:
        return
    try:
        mod = types.ModuleType("antenv.axon_hooks")
        _h = [None]
        mod.set_axon_ntff_profile_hook = lambda h: _h.__setitem__(0, h)
        mod.get_axon_ntff_profile_hook = lambda: _h[0]
        sys.modules["antenv.axon_hooks"] = mod
        import antenv

        antenv.axon_hooks = mod
        if "/root/.axon_site" not in sys.path:
            sys.path.insert(0, "/root/.axon_site")
        from trn_agent_boot.trn_boot import _ntff_profile_via_ctypes

        mod.set_axon_ntff_profile_hook(
            _ntff_profile_via_ctypes("/opt/axon/libaxon_pjrt.so")
        )
    except Exception:
        pass


def build_program(n_layers=L, taps=()):
    nc = bacc.Bacc("TRN2", target_bir_lowering=False, debug=False,
                   num_devices=N_CORES)

    d = {}
    d["x0r"] = nc.dram_tensor("x0r", (D, S), F32R, kind="ExternalInput").ap()
    d["x0b"] = nc.dram_tensor("x0b", (D, S), BF16, kind="ExternalInput").ap()
    d["wqkv"] = nc.dram_tensor("wqkv", (n_layers, 128, KC * 3 * D), BF16,
                               kind="ExternalInput").ap()
    d["wproj"] = nc.dram_tensor("wproj", (n_layers, 128, KC * D), BF16,
                                kind="ExternalInput").ap()
    d["wfca"] = nc.dram_tensor("wfca", (n_layers, 128, KC * D), BF16,
                               kind="ExternalInput").ap()
    d["wfcb"] = nc.dram_tensor("wfcb", (n_layers, KF - KC, 128, KC * 128),
                               BF16, kind="ExternalInput").ap()
    d["wpr"] = nc.dram_tensor("wpr", (n_layers, DF, D), BF16,
                              kind="ExternalInput").ap()
    # biases / gains pre-transposed on host to [L, 128, n]
    d["bqkv"] = nc.dram_tensor("bqkv", (n_layers, 128, 3 * D // 128), F32,
                               kind="ExternalInput").ap()
    d["bv"] = nc.dram_tensor("bv", (n_layers, D), BF16, kind="ExternalInput").ap()
    d["bproj"] = nc.dram_tensor("bproj", (n_layers, 128, KC), F32,
                                kind="ExternalInput").ap()
    d["bfc"] = nc.dram_tensor("bfc", (n_layers, 128, KF), F32,
                              kind="ExternalInput").ap()
    d["bpr"] = nc.dram_tensor("bpr", (n_layers, 128, KC), F32,
                              kind="ExternalInput").ap()
    for nm in ("g1", "b1", "g2", "b2"):
        d[nm] = nc.dram_tensor(nm, (n_layers, 128, KC), F32,
                               kind="ExternalInput").ap()
    d["triu"] = nc.dram_tensor("triu", (128, 128), BF16, kind="ExternalInput").ap()
    d["ones2d_s"] = nc.dram_tensor("ones2d_s", (128, 128), F32R,
                                   kind="ExternalInput").ap()
    d["ones_red_s"] = nc.dram_tensor("ones_red_s", (128, 2), F32R,
                                     kind="ExternalInput").ap()
    d["ones1"] = nc.dram_tensor("ones1", (1, 128), F32R, kind="ExternalInput").ap()
    d["sel1"] = nc.dram_tensor("sel1", (1, 64), F32R,
                               kind="ExternalInput").ap()
    d["out"] = nc.dram_tensor("out", (D, S), F32R, kind="ExternalOutput").ap()
    d["warmo"] = nc.dram_tensor("warmo", (1, 1), F32, kind="ExternalOutput").ap()
    d["_taps"] = {}
    for tp in taps:
        if tp.startswith("v"):
            shape = (SC, 128, H, 66)
        else:
            shape = (KC, 128, S)
        d["_taps"][tp] = nc.dram_tensor(
            f"tap_{tp}", shape, BF16 if tp in ("q", "k", "v", "a", "n1") else F32R,
            kind="ExternalOutput").ap()

    with tile.TileContext(nc) as tc, \
         nc.allow_low_precision(reason="bf16 matmul datapath; rel-err budget 2e-2"):
        _emit(tc, nc, n_layers, d)
    nc.compile()
    return nc


def _tap(nc, d, name, tiles):
    if name in d["_taps"]:
        for i, t in enumerate(tiles):
            nc.sync.dma_start(out=d["_taps"][name][i], in_=t)


def _emit(tc, nc, n_layers, d):
    ctx = contextlib.ExitStack()

    consts = ctx.enter_context(tc.tile_pool(name="consts", bufs=1))
    persist = ctx.enter_context(tc.tile_pool(name="persist", bufs=1))
    resid = ctx.enter_context(tc.tile_pool(name="resid", bufs=2))
    xb_pool = ctx.enter_context(tc.tile_pool(name="xb", bufs=2))
    wq_pool = ctx.enter_context(tc.tile_pool(name="wq", bufs=1))
    wp_pool = ctx.enter_context(tc.tile_pool(name="wp", bufs=1))
    wf_pool = ctx.enter_context(tc.tile_pool(name="wf", bufs=1))
    wr_pool = ctx.enter_context(tc.tile_pool(name="wr", bufs=4))
    wfb_pool = ctx.enter_context(tc.tile_pool(name="wfb", bufs=4))
    probs_pool = ctx.enter_context(tc.tile_pool(name="probs", bufs=2))
    gelu_pool = ctx.enter_context(tc.tile_pool(name="gelu", bufs=8))
    stats_pool = ctx.enter_context(tc.tile_pool(name="stats", bufs=2))
    small_pool = ctx.enter_context(tc.tile_pool(name="small", bufs=2))
    lnsm_pool = ctx.enter_context(tc.tile_pool(name="lnsm", bufs=1))
    lnbc_pool = ctx.enter_context(tc.tile_pool(name="lnbc", bufs=2))
    bias_pool = ctx.enter_context(tc.tile_pool(name="bias", bufs=2))

    cn = {}
    for nm, shape, dt_ in (
        ("triu", [128, 128], BF16),
        ("ones2d_s", [128, 128], F32R),
        ("ones_red_s", [128, 2], F32R),
        ("ones1", [1, 128], F32R),
        ("sel1", [1, 64], F32R),
    ):
        cn[nm] = consts.tile(shape, dt_, tag=nm, name=nm)
        nc.sync.dma_start(out=cn[nm], in_=d[nm])
    cn["eps"] = consts.tile([1, 1], F32, tag="eps", name="eps")
    nc.vector.memset(cn["eps"], EPS)
    # warm-up chain for ACT table preloads: each op reads+writes this tile
    # so the chain stays live (a dead output would be DCE'd by walrus)
    cn["warm"] = consts.tile([1, 1], F32, tag="warm", name="warm")
    nc.vector.memset(cn["warm"], 0.25)


    # v in natural layout [seq, head, dh] with a trailing ones column:
    # lhsT slice [:, h, 1:66] -> out rows 0:63 = v, row 64 = sum(probs)
    # (the softmax denominator rides the AV matmul for free).
    v_nat = [persist.tile([128, H, 66], BF16, tag=f"vn{c}", name=f"vn{c}")
             for c in range(SC)]
    for c in range(SC):
        nc.gpsimd.memset(v_nat[c][:, :, 65:66], 1.0)

    qT = [persist.tile([128, S], BF16, tag=f"q{i}", name=f"qT{i}")
          for i in range(KC)]
    kT = [persist.tile([128, S], BF16, tag=f"k{i}", name=f"kT{i}")
          for i in range(KC)]
    aT = [persist.tile([128, S], BF16, tag=f"a{i}", name=f"aT{i}")
          for i in range(KC)]

    pools = dict(resid=resid, xb=xb_pool, wq=wq_pool, wp=wp_pool, wf=wf_pool,
                 wr=wr_pool, wfb=wfb_pool, probs=probs_pool, gelu=gelu_pool,
                 stats=stats_pool, small=small_pool, lnsm=lnsm_pool,
                 lnbc=lnbc_pool, bias=bias_pool, persist=persist,
                 v_nat=v_nat, qT=qT, kT=kT, aT=aT)

    # layer 0 inputs + weights
    wq_t = wq_pool.tile([128, KC * 3 * D], BF16, tag="wqkv", name="wq0")
    nc.sync.dma_start(out=wq_t, in_=d["wqkv"][0])
    wfc_t = wf_pool.tile([128, KC * D], BF16, tag="wfca", name="wfca0")
    nc.sync.dma_start(out=wfc_t, in_=d["wfca"][0])

    xb = [xb_pool.tile([128, S], BF16, tag=f"xb{k}", name=f"xb0_{k}")
          for k in range(KC)]
    xr = [resid.tile([128, S], F32R, tag=f"r{k}", name=f"xr0_{k}")
          for k in range(KC)]
    for k in range(KC):
        nc.sync.dma_start(out=xb[k], in_=d["x0b"][128 * k:128 * (k + 1), :])
        nc.sync.dma_start(out=xr[k], in_=d["x0r"][128 * k:128 * (k + 1), :])

    feed = None   # per-chunk emit callbacks from previous layer's LN2
    for l in range(n_layers):
        with nc.named_scope(f"layer{l}"):
            wq_t, wfc_t, xb, xr, feed = _layer(
                tc, nc, l, n_layers, wq_t, wfc_t, xb, xr, feed, d, cn, pools)

    # final LN2 apply chunks (last layer's feed) then store
    for k in range(KC):
        feed[k]()
        nc.sync.dma_start(out=d["out"][128 * k:128 * (k + 1), :], in_=xr[k])
    nc.sync.dma_start(out=d["warmo"], in_=cn["warm"])
    ctx.close()


class _LNState:
    """LayerNorm over the partition (feature) axis, pipelined:
    accum() per chunk (PE reductions), finish() (var chain + SBUF
    snapshots of mean/rsd broadcasts), apply() per chunk
    (normalize+affine, dual bf16/f32r outputs).

    All PSUM pools open and close inside finish()/__init__, strictly
    nested within the caller's enclosing PSUM phase pool, so the tile
    allocator's stack discipline holds.  apply() reads only SBUF.
    """

    def __init__(self, tc, nc, tag, cn, stats, lnsm, lnbc):
        self.tc, self.nc, self.cn = tc, nc, cn
        self.stats, self.small, self.lnbc, self.tag = stats, lnsm, lnbc, tag
        self.ctx = contextlib.ExitStack()
        ps_mu = self.ctx.enter_context(
            tc.tile_pool(name=f"{tag}_mu", bufs=1, space="PSUM"))
        self._psq_ctx = contextlib.ExitStack()
        ps_q = self._psq_ctx.enter_context(
            tc.tile_pool(name=f"{tag}_q", bufs=1, space="PSUM"))
        self.mu = ps_mu.tile([128, S], F32, tag="mu", name=f"{tag}_mu")
        self.psq = ps_q.tile([2, S], F32, tag="sq", name=f"{tag}_sq")

    def accum(self, src, k):
        nc, cn = self.nc, self.cn
        sq = self.stats.tile([128, S], F32R, tag="sq", name=f"{self.tag}sq{k}")
        nc.gpsimd.tensor_tensor(out=sq, in0=src, in1=src, op=OP.mult)
        nc.tensor.matmul(self.mu, cn["ones2d_s"], src,
                         start=(k == 0), stop=(k == KC - 1))
        nc.tensor.matmul(self.psq, cn["ones_red_s"], sq,
                         start=(k == 0), stop=(k == KC - 1))

    def finish(self, ext_prs=False):
        nc, cn = self.nc, self.cn
        var = self.small.tile([1, S], F32, tag="var", name=f"{self.tag}var")
        nc.scalar.activation(out=var, in_=self.mu[0:1, :], func=AF.Square)
        v2 = self.small.tile([1, S], F32, tag="v2", name=f"{self.tag}v2")
        nc.vector.tensor_tensor(out=v2, in0=self.psq[0:1, :], in1=var,
                                op=OP.subtract)
        # sqrt table was pre-warmed by a dummy activation, so this is cheap
        sd = self.small.tile([1, S], F32, tag="var", name=f"{self.tag}sd")
        nc.scalar.activation(out=sd, in_=v2, func=AF.Sqrt, bias=cn["eps"])
        rc = self.small.tile([1, S], F32, tag="v2", name=f"{self.tag}rc")
        nc.vector.reciprocal_approx_fast(out=rc, in_=sd)
        rsd = self.small.tile([1, S], F32R, tag="rsd", name=f"{self.tag}rsd")
        nc.vector.tensor_copy(out=rsd, in_=rc)
        self.mu_s = self.lnbc.tile([128, S], F32, tag="mu_s",
                                   name=f"{self.tag}_mus")
        nc.scalar.activation(out=self.mu_s, in_=self.mu, func=AF.Identity)
        self._psq_ctx.close()          # free the psq bank
        self.rsd = rsd
        if not ext_prs:
            with self.tc.tile_pool(name=f"{self.tag}_prs", bufs=1,
                                   space="PSUM") as ps_prs:
                prs = ps_prs.tile([128, S], F32, tag="prs",
                                  name=f"{self.tag}_prs")
                nc.tensor.matmul(prs, cn["ones1"], rsd, start=True, stop=True)
                self.prs_s = self.lnbc.tile([128, S], F32, tag="prs_s",
                                            name=f"{self.tag}_prss")
                nc.scalar.activation(out=self.prs_s, in_=prs,
                                     func=AF.Identity)
        self.ctx.close()               # free the mu bank

    def set_prs(self, prs):
        """Emit the rsd broadcast into a caller-owned PSUM tile; apply()
        then reads it directly (saves the SBUF snapshot copy)."""
        self.nc.tensor.matmul(prs, self.cn["ones1"], self.rsd,
                              start=True, stop=True)
        self.prs_s = prs

    def apply(self, k, src, g_t, b_t, dst_bf, dst_r):
        nc = self.nc
        t = self.stats.tile([128, S], F32, tag="t", name=f"{self.tag}t{k}")
        nc.gpsimd.tensor_tensor(out=t, in0=src, in1=self.mu_s,
                                op=OP.subtract)
        u = self.stats.tile([128, S], F32, tag="u", name=f"{self.tag}u{k}")
        nc.vector.scalar_tensor_tensor(out=u, in0=t, scalar=g_t[:, k:k + 1],
                                       in1=self.prs_s, op0=OP.mult,
                                       op1=OP.mult)
        nc.scalar.activation(out=dst_bf, in_=u, func=AF.Identity,
                             bias=b_t[:, k:k + 1], scale=1.0)
        nc.scalar.activation(out=dst_r, in_=u, func=AF.Identity,
                             bias=b_t[:, k:k + 1], scale=1.0)


def _layer(tc, nc, l, n_layers, wq_t, wfc_t, xb, xr, feed, d, cn, pools):
    bias_pool = pools["bias"]
    stats = pools["stats"]
    small = pools["small"]
    lnsm = pools["lnsm"]
    v_nat, qT, kT, aT = pools["v_nat"], pools["qT"], pools["kT"], pools["aT"]

    # ---- layer-start weight/bias DMAs (wproj single-buffered: its
    # previous-layer reads finished phases ago, so this runs during A). ----
    wproj_t = pools["wp"].tile([128, KC * D], BF16, tag="wproj")
    nc.sync.dma_start(out=wproj_t, in_=d["wproj"][l])

    def ld_bias(name, width):
        t = bias_pool.tile([128, width], F32, tag=name, name=f"{name}{l}")
        nc.sync.dma_start(out=t, in_=d[name][l])
        return t

    bqkv_t = ld_bias("bqkv", 3 * D // 128)
    bproj_t = ld_bias("bproj", KC)
    g1_t = ld_bias("g1", KC)
    b1_t = ld_bias("b1", KC)
    bfc_t = ld_bias("bfc", KF)
    bpr_t = ld_bias("bpr", KC)
    g2_t = ld_bias("g2", KC)
    b2_t = ld_bias("b2", KC)
    bv_b = bias_pool.tile([128, D], BF16, tag="bvb", name=f"bvb{l}")
    nc.sync.dma_start(out=bv_b, in_=d["bv"][l].partition_broadcast(128))
    bv_h = bv_b.rearrange("p (h e) -> p h e", e=DH)

    # =====================================================================
    # Phase A.  wave0: k-outer over x chunks as LN2(l-1) produces them:
    # v half0 (4 psum groups) + qkv oc 0,1 (2 groups) = 6 banks, plus the
    # still-live LN2 mu/prs banks of the previous layer = 8.
    # =====================================================================
    VOFF = 1536  # column offset of v inside wqkv's 2304-wide block

    # wave0: all 8 v psum groups (LN2(l-1) psum pools are fully closed, so
    # all 8 banks are free); 3072 rows per x-chunk matches the feed rate
    with tc.tile_pool(name="ps_w0", bufs=1, space="PSUM") as ps_w0:
        pv0 = [ps_w0.tile([128, 384], F32, tag=f"pv0_{j}", name=f"pv0_{j}")
               for j in range(2 * SC)]
        for k in range(KC):
            if feed is not None:
                feed[k]()          # emit LN2(l-1) apply for chunk k
            for sc in range(SC):
                for half in range(2):
                    nc.tensor.matmul(
                        pv0[2 * sc + half], xb[k][:, 128 * sc:128 * (sc + 1)],
                        wq_t[:, k * 2304 + VOFF + 384 * half:
                             k * 2304 + VOFF + 384 * (half + 1)],
                        start=(k == 0), stop=(k == KC - 1))
        for sc in range(SC):
            for half in range(2):
                nc.vector.tensor_tensor(
                    out=v_nat[sc][:, 6 * half:6 * half + 6, 1:65],
                    in0=pv0[2 * sc + half].rearrange("p (h e) -> p h e", e=DH),
                    in1=bv_h[:, 6 * half:6 * half + 6, :], op=OP.add)

    # wave1: all q/k output chunks; kT chunks early so phase B starts
    with tc.tile_pool(name="ps_qk", bufs=3, space="PSUM") as ps_qk:
        for oc in [6, 0, 7, 1, 8, 2, 9, 3, 10, 4, 11, 5]:
            pt = ps_qk.tile([128, S], F32, tag="qk")
            for k in range(KC):
                nc.tensor.matmul(
                    pt,
                    wq_t[:, k * 2304 + 128 * oc:k * 2304 + 128 * (oc + 1)],
                    xb[k], start=(k == 0), stop=(k == KC - 1))
            dst = qT[oc] if oc < KC else kT[oc - KC]
            nc.vector.tensor_scalar(out=dst, in0=pt,
                                    scalar1=bqkv_t[:, oc:oc + 1],
                                    scalar2=None, op0=OP.add)

    if l == 0:
        _tap(nc, d, "q", qT)
        _tap(nc, d, "k", kT)
        _tap(nc, d, "v", v_nat)

    # =====================================================================
    # Phase B: attention; softmax denominator rides row 64 of pav, is
    # DMA-gathered into den12, and pairs share one reciprocal.
    # =====================================================================
    with tc.tile_pool(name="ps_sc", bufs=3, space="PSUM") as ps_sc, \
         tc.tile_pool(name="ps_av", bufs=2, space="PSUM") as ps_av, \
         tc.tile_pool(name="ps_dn", bufs=2, space="PSUM") as ps_dn, \
         tc.tile_pool(name="ps_bc", bufs=1, space="PSUM") as ps_bc:

        pavs = {}

        def emit_head(h):
            hc, hh = h // 2, (h % 2) * 64
            probs = [pools["probs"].tile([128, S], BF16, tag=f"pb{c}",
                                         name=f"pb{c}_{h}") for c in range(SC)]
            for c in range(SC):
                n0 = 128 * c
                pt = ps_sc.tile([128, S], F32, tag="score")
                nc.tensor.matmul(pt[:, 0:S - n0],
                                 kT[hc][hh:hh + 64, n0:n0 + 128],
                                 qT[hc][hh:hh + 64, n0:S],
                                 start=True, stop=True)
                nc.scalar.activation(out=probs[c][:, n0:S],
                                     in_=pt[:, 0:S - n0],
                                     func=AF.Exp, scale=0.125)
                nc.gpsimd.tensor_tensor(
                    out=probs[c][:, n0:n0 + 128],
                    in0=probs[c][:, n0:n0 + 128],
                    in1=cn["triu"], op=OP.mult)
            pav = ps_av.tile([64, S], F32, tag="pav", name=f"pav{h}")
            pden = ps_dn.tile([1, S], F32, tag="pden", name=f"pden{h}")
            pavs[h] = (pav, pden)
            for c in range(SC):
                n0 = 128 * c
                nc.tensor.matmul(pav[0:64, n0:S], v_nat[c][:, h, 1:65],
                                 probs[c][:, n0:S], start=(c == 0),
                                 stop=(c == SC - 1), skip_group_check=True)
                nc.tensor.matmul(pden[0:1, n0:S], v_nat[c][:, h, 65:66],
                                 probs[c][:, n0:S], start=(c == 0),
                                 stop=(c == SC - 1), skip_group_check=True)

        def emit_norm(h):
            hc = h // 2
            pav, pden = pavs.pop(h)
            rec = small.tile([1, S], F32, tag="rec", name=f"rec{h}")
            nc.vector.reciprocal_approx_fast(out=rec, in_=pden)
            rec_r = small.tile([1, S], F32R, tag="recr", name=f"recr{h}")
            nc.vector.tensor_copy(out=rec_r, in_=rec)
            pbc = ps_bc.tile([64, S], F32, tag="pbc")
            nc.tensor.matmul(pbc, cn["sel1"], rec_r, start=True, stop=True)
            bc_s = small.tile([64, S], F32, tag="bcs", name=f"bcs{h}")
            nc.vector.tensor_copy(out=bc_s, in_=pbc)
            if h % 2 == 0:
                nc.vector.tensor_tensor(out=aT[hc][0:64, :],
                                        in0=pav[0:64, :], in1=bc_s,
                                        op=OP.mult)
            else:
                att = small.tile([64, S], BF16, tag="att", name=f"att{h}")
                nc.vector.tensor_tensor(out=att, in0=pav[0:64, :],
                                        in1=bc_s, op=OP.mult)
                nc.sync.dma_start(out=aT[hc][64:128, :], in_=att)

        for h in range(H):
            if h >= 1:
                emit_norm(h - 1)
            emit_head(h)
        emit_norm(H - 1)
        # warm the sqrt table while the PE runs proj (anchored on aT so the
        # scheduler can't hoist it; bias chains warm's liveness)
        nc.scalar.activation(out=cn["warm"], in_=aT[5][0:1, 0:1],
                             func=AF.Sqrt, bias=cn["warm"])

    # prefetch next layer's wqkv now: the burst runs during C/LN1, clear of
    # phase B's latency-critical small DMAs
    wq_next = None
    if l + 1 < n_layers:
        wq_next = pools["wq"].tile([128, KC * 3 * D], BF16, tag="wqkv",
                                   name=f"wq{l + 1}")
        nc.sync.dma_start(out=wq_next, in_=d["wqkv"][l + 1])

    if l == 0:
        _tap(nc, d, "a", aT)

    # =====================================================================
    # Phase C: attn out proj; residual+bias via one DVE STT; LN1 stats
    # interleaved so the PE stays busy into the LN tail.
    # =====================================================================
    res1 = [pools["resid"].tile([128, S], F32R, tag=f"r{k}", name=f"res1_{k}")
            for k in range(KC)]

    with tc.tile_pool(name="ps_pj", bufs=3, space="PSUM") as ps_pj:
        ln1 = _LNState(tc, nc, "ln1", cn, stats, lnsm, pools["lnbc"])
        for oc in range(KC):
            pt = ps_pj.tile([128, S], F32, tag="pj")
            for k in range(KC):
                nc.tensor.matmul(
                    pt, wproj_t[:, k * D + 128 * oc:k * D + 128 * (oc + 1)],
                    aT[k], start=(k == 0), stop=(k == KC - 1))
            nc.vector.scalar_tensor_tensor(out=res1[oc], in0=pt,
                                           scalar=bproj_t[:, oc:oc + 1],
                                           in1=xr[oc], op0=OP.add, op1=OP.add)
            ln1.accum(res1[oc], oc)
        ln1.finish(ext_prs=True)
        # warm the gelu table while the PE runs the fc wave
        nc.scalar.activation(out=cn["warm"], in_=ln1.rsd[0:1, 0:1],
                             func=AF.Gelu_apprx_tanh, bias=cn["warm"])
    if l == 0:
        _tap(nc, d, "r1", res1)

    # =====================================================================
    # Phase D.  LN1 apply feeds fc wave0 (k-outer, 6 psum groups), then
    # fc(kf)/pr(kf-6) interleaved so the PE never waits on gelu.
    # =====================================================================
    nT_b = [pools["xb"].tile([128, S], BF16, tag=f"xb{k}", name=f"nTb{k}")
            for k in range(KC)]
    nT_r = [pools["resid"].tile([128, S], F32R, tag=f"r{k}", name=f"nTr{k}")
            for k in range(KC)]
    gks = [pools["gelu"].tile([128, S], BF16, tag="gk", name=f"gk{kf}")
           for kf in range(KF)]

    with tc.tile_pool(name="ps_f0", bufs=1, space="PSUM") as ps_f0:
        pf0 = [ps_f0.tile([128, S], F32, tag=f"pf{j}", name=f"pf{j}")
               for j in range(KC)]
        ln1_prs = ps_f0.tile([128, S], F32, tag="ln1prs", name="ln1prs")
        ln1.set_prs(ln1_prs)
        for k in range(KC):
            ln1.apply(k, res1[k], g1_t, b1_t, nT_b[k], nT_r[k])
            for j in range(KC):
                nc.tensor.matmul(
                    pf0[j], wfc_t[:, k * D + 128 * j:k * D + 128 * (j + 1)],
                    nT_b[k], start=(k == 0), stop=(k == KC - 1))
        for j in range(KC):
            nc.scalar.activation(out=gks[j], in_=pf0[j],
                                 func=AF.Gelu_apprx_tanh,
                                 bias=bfc_t[:, j:j + 1], scale=1.0)
    if l == 0:
        _tap(nc, d, "n1", nT_b)

    res2 = [pools["resid"].tile([128, S], F32R, tag=f"r{k}", name=f"res2_{k}")
            for k in range(KC)]
    ln2_box = [None]

    with tc.tile_pool(name="ps_pr", bufs=1, space="PSUM") as ps_pr:
        pr_acc = [ps_pr.tile([128, S], F32, tag=f"pr{oc}", name=f"pr{oc}")
                  for oc in range(KC)]
        wprs = {}

        def dma_wpr(kf):
            wpr_k = pools["wr"].tile([128, D], BF16, tag="wprk")
            nc.sync.dma_start(out=wpr_k,
                              in_=d["wpr"][l, 128 * kf:128 * (kf + 1), :])
            wprs[kf] = wpr_k

        wfbs = {}

        def dma_wfcb(kf):
            wfb = pools["wfb"].tile([128, KC * 128], BF16, tag="wfcb")
            nc.sync.dma_start(out=wfb, in_=d["wfcb"][l, kf - KC])
            wfbs[kf] = wfb

        def emit_pr(kf, tail=False):
            wpr_k = wprs.pop(kf)
            for oc in range(KC):
                nc.tensor.matmul(pr_acc[oc],
                                 wpr_k[:, 128 * oc:128 * (oc + 1)],
                                 gks[kf], start=(kf == 0),
                                 stop=(kf == KF - 1))
                if tail:
                    nc.vector.scalar_tensor_tensor(
                        out=res2[oc], in0=pr_acc[oc],
                        scalar=bpr_t[:, oc:oc + 1], in1=nT_r[oc],
                        op0=OP.add, op1=OP.add)
                    ln2_box[0].accum(res2[oc], oc)

        with tc.tile_pool(name="ps_fc", bufs=2, space="PSUM") as ps_fc:
            for j in range(4):
                dma_wpr(j)
            for j in range(4):
                dma_wfcb(KC + j)
            for kf in range(KC, KF):
                wfb = wfbs.pop(kf)
                pf = ps_fc.tile([128, S], F32, tag="fc")
                for k in range(KC):
                    nc.tensor.matmul(
                        pf, wfb[:, 128 * k:128 * (k + 1)],
                        nT_b[k], start=(k == 0), stop=(k == KC - 1))
                nc.scalar.activation(out=gks[kf], in_=pf,
                                     func=AF.Gelu_apprx_tanh,
                                     bias=bfc_t[:, kf:kf + 1], scale=1.0)
                emit_pr(kf - KC)
                dma_wpr(kf - 2)
                if kf + 4 < KF:
                    dma_wfcb(kf + 4)
            # warm the sqrt table while the PE runs the pr tail (anchored on
            # the last gelu output so it can't run before the gelu stream)
            nc.scalar.activation(out=cn["warm"], in_=gks[KF - 1][0:1, 0:1],
                                 func=AF.Sqrt, bias=cn["warm"])
        for kf in range(KF - KC, KF - 1):
            if kf + 4 < KF:
                dma_wpr(kf + 4)
            emit_pr(kf)
        # ps_fc closed -> 2 banks free for LN2 mu/psq during the tail
        ln2_box[0] = _LNState(tc, nc, "ln2", cn, stats, lnsm, pools["lnbc"])
        emit_pr(KF - 1, tail=True)
        ln2_box[0].finish()
        # warm the exp table for the next layer's softmax (anchored on the
        # LN2 sqrt-chain output so it cannot evict the sqrt table early)
        nc.scalar.activation(out=cn["warm"], in_=ln2_box[0].prs_s[0:1, 0:1],
                             func=AF.Exp, bias=cn["warm"])
    ln2 = ln2_box[0]
    if l == 0:
        _tap(nc, d, "r2", res2)

    # prefetch next layer's wfc at the END of the layer: the 13us burst
    # lands behind this layer's wpr stream and runs during LN2/A/B of l+1
    wfc_next = None
    if l + 1 < n_layers:
        wfc_next = pools["wf"].tile([128, KC * D], BF16, tag="wfca",
                                    name=f"wfca{l + 1}")
        nc.sync.dma_start(out=wfc_next, in_=d["wfca"][l + 1])

    xb_new = [pools["xb"].tile([128, S], BF16, tag=f"xb{k}", name=f"xbn{k}")
              for k in range(KC)]
    xr_new = [pools["resid"].tile([128, S], F32R, tag=f"r{k}", name=f"xrn{k}")
              for k in range(KC)]

    def make_feed(k):
        def f():
            ln2.apply(k, res2[k], g2_t, b2_t, xb_new[k], xr_new[k])
        return f

    feed_new = [make_feed(k) for k in range(KC)]
    return wq_next, wfc_next, xb_new, xr_new, feed_new


# =========================================================================
# Host side
# =========================================================================
_CACHE = {}


def _get_program():
    if "nc" not in _CACHE:
        _install_ntff_hook()
        _CACHE["nc"] = build_program(L)
    return _CACHE["nc"]


def make_in_maps(inputs, n_layers=L):
    import ml_dtypes
    bf16 = ml_dtypes.bfloat16

    tokens = np.asarray(inputs["tokens"])
    we = np.asarray(inputs["we"], dtype=np.float32)
    pos = we[V:V + S]                                  # [S, D]

    def f32(name):
        return np.ascontiguousarray(np.asarray(inputs[name])[:n_layers],
                                    dtype=np.float32)

    def pack(w, n):
        # [L, D, n] -> [L, 128, KC*n]: a[l, p, k*n + j] = w[l, 128k+p, j]
        return np.ascontiguousarray(
            w.reshape(n_layers, KC, 128, n).transpose(0, 2, 1, 3).reshape(
                n_layers, 128, KC * n)).astype(bf16)

    def pack_fca(w):
        # [L, D, DF] -> [L, 128, KC*D]: a[l,p,k*768+j*128+c] = w[l,128k+p,128j+c], j<6
        return np.ascontiguousarray(
            w[:, :, :D].reshape(n_layers, KC, 128, D).transpose(
                0, 2, 1, 3).reshape(n_layers, 128, KC * D)).astype(bf16)

    def pack_fcb(w):
        # [L, D, DF] -> [L, KF-KC, 128, KC*128]:
        # b[l, kf-6, p, k*128+c] = w[l, 128k+p, 128kf+c]
        t = w[:, :, D:].reshape(n_layers, KC, 128, KF - KC, 128)
        return np.ascontiguousarray(
            t.transpose(0, 3, 2, 1, 4).reshape(
                n_layers, KF - KC, 128, KC * 128)).astype(bf16)

    def bias_t(b, n):
        # [L, n*128] -> [L, 128, n]
        return np.ascontiguousarray(
            b.reshape(n_layers, n, 128).transpose(0, 2, 1))

    bqkv = f32("bqkv")
    shared = {
        "wqkv": pack(f32("wqkv"), 3 * D),
        "wproj": pack(f32("wproj"), D),
        "wfca": pack_fca(f32("wfc")),
        "wfcb": pack_fcb(f32("wfc")),
        "wpr": np.ascontiguousarray(f32("wpr")).astype(bf16),
        "bqkv": bias_t(bqkv, 3 * D // 128),
        "bv": np.ascontiguousarray(bqkv[:, 2 * D:3 * D]).astype(bf16),
        "bproj": bias_t(f32("bproj"), KC),
        "bfc": bias_t(f32("bfc"), KF),
        "bpr": bias_t(f32("bpr"), KC),
        "g1": bias_t(f32("g1"), KC),
        "b1": bias_t(f32("b1"), KC),
        "g2": bias_t(f32("g2"), KC),
        "b2": bias_t(f32("b2"), KC),
        "triu": np.triu(np.ones((128, 128), np.float32)).astype(bf16),
        "ones2d_s": np.full((128, 128), 1.0 / D, np.float32),
        "ones1": np.ones((1, 128), np.float32),
    }
    shared["sel1"] = np.ones((1, 64), np.float32)
    ones_red = np.zeros((128, 2), np.float32)
    ones_red[:, 0] = 1.0 / D
    shared["ones_red_s"] = ones_red

    in_maps = []
    for b in range(N_CORES):
        x0 = we[tokens[b]] + pos                       # [S, D]
        x0T = np.ascontiguousarray(x0.T, dtype=np.float32)
        m = dict(shared)
        m["x0r"] = x0T
        m["x0b"] = x0T.astype(bf16)
        in_maps.append(m)
    return in_maps


def run(inputs, trace=False):
    nc = _get_program()
    in_maps = make_in_maps(inputs)
    res = bass_utils.run_bass_kernel_spmd(nc, in_maps,
                                          core_ids=list(range(N_CORES)),
                                          trace=trace)
    outs = np.stack([np.asarray(res.results[b]["out"]).T
                     for b in range(N_CORES)])
    return outs.astype(np.float32), res


def kernel(**inputs):
    out, _ = run(inputs, trace=False)
    return out
